# revision 43
# baseline (speedup 1.0000x reference)
"""Trainium2 Bass kernel for nn_AliasFreeActivation (alias-free GAN activation).

Pipeline per (n, c) plane X [64, 64]:
    y = Wdn.T @ ( sqrt(2) * lrelu_0.2( Wup.T @ (X + b) @ Wup ) ) @ Wdn
where Wup [64, 128] / Wdn [128, 64] are the upfirdn band matrices for the
separable 12-tap filter (up=2 / down=2), built on host.

Device mapping (fused matmul chain — zero transposes):
    M1  t1 = x_aug.T @ Wup_aug    [2 planes: 128(2w), 128h']   (data as lhsT)
    M2  u^T = Wup.T @ t1_p        [128w', 128h']               (filter stationary)
    L   s = lrelu(sqrt2 * u)      ACT Lrelu alpha=0.2, PSUM->SBUF bf16
    M3  v = s.T @ Wdn             [128h', 64w'']               (data as lhsT)
    M4  y = Wdn.T @ v_all         [64h'', 512]                 (one matmul / group)

Sharding: pure data parallel over batch: core i gets input[i] -> [512, 64, 64].
Each core processes 64 groups of 8 channel-planes.
"""

import os
import sys

for _p in ("/opt/trn_rl_repo", "/opt/pypackages"):
    if _p not in sys.path:
        sys.path.append(_p)

import numpy as np
import ml_dtypes

N_CORES = 8
B, C, H, W = 8, 512, 64, 64
GROUP = 8                 # channel planes per group
N_GROUPS = C // GROUP     # 64
DMA_BATCH = 4             # groups per DMA transfer
UP_LEN = 128
NEG_SLOPE = 0.2
SQRT2 = float(2.0 ** 0.5)

# 12-tap hann-windowed-sinc lowpass, as in the reference module
_FILT = np.array([0.0, 0.00398, -0.01884, -0.05155, 0.12443, 0.44197,
                  0.44197, 0.12443, -0.05155, -0.01884, 0.00398, 0.0],
                 dtype=np.float64)
_FILT = _FILT / _FILT.sum()

_BF16 = ml_dtypes.bfloat16

_LAST_RESULT = None   # BassKernelResults of the most recent run (for test.py)
_CACHED = None        # (nc, meta) cache so repeat kernel() calls skip rebuild


def _upfirdn_matrix(k, L, up, down, pad0, pad1):
    """Band matrix Wf such that y = x @ Wf applies upfirdn along an axis."""
    K = len(k)
    Ld = (L - 1) * up + 1
    n_out = (Ld + pad0 + (pad1 + up - 1) - K) // down + 1
    Wf = np.zeros((L, n_out), dtype=np.float64)
    for j in range(n_out):
        for t in range(K):
            m = j * down + t - pad0
            if 0 <= m < Ld and m % up == 0:
                Wf[m // up, j] += k[K - 1 - t]
    return Wf


def _build_consts(up_filter, down_filter):
    k_up = np.asarray(up_filter, dtype=np.float64) * 2.0   # prescaled by UP
    k_dn = np.asarray(down_filter, dtype=np.float64)
    Wup = _upfirdn_matrix(k_up, 64, 2, 1, 6, 5)            # [64, 128]
    Wdn = _upfirdn_matrix(k_dn, 128, 1, 2, 5, 5)           # [128, 64]
    return Wup.astype(_BF16), Wdn.astype(_BF16)


def _build_bass(n_groups=N_GROUPS, repeat=1):
    import concourse.bacc as bacc
    import concourse.mybir as mybir
    from concourse.tile import TileContext

    f32 = mybir.dt.float32
    bf16 = mybir.dt.bfloat16

    nc = bacc.Bacc("TRN2", target_bir_lowering=False)

    # x / out live in DRAM as [H, C*W] (host pre/post-transposes) so every
    # DMA is a plain 2-D slice with 512*dsize contiguous bytes per partition.
    x = nc.dram_tensor("x", [H, C * W], bf16, kind="ExternalInput")
    wup = nc.dram_tensor("wup", [64, UP_LEN], bf16, kind="ExternalInput")
    wdn = nc.dram_tensor("wdn", [UP_LEN, W], bf16, kind="ExternalInput")
    out = nc.dram_tensor("out", [H, C * W], f32, kind="ExternalOutput")

    with TileContext(nc) as tc:
        with (
            tc.tile_pool(name="consts", bufs=1) as cpool,
            tc.tile_pool(name="xt", bufs=int(os.environ.get("XB", 3))) as xpool,
            tc.tile_pool(name="t1ps", bufs=int(os.environ.get("T1B", 2)), space="PSUM") as t1ps_pool,
            tc.tile_pool(name="t1sb", bufs=int(os.environ.get("T1SB", 4))) as t1sb_pool,
            tc.tile_pool(name="ups", bufs=int(os.environ.get("UPB", 1)), space="PSUM") as ups_pool,
            tc.tile_pool(name="ssb", bufs=int(os.environ.get("SSB", 4))) as ssb_pool,
            tc.tile_pool(name="vps", bufs=int(os.environ.get("VPB", 1)), space="PSUM") as vps_pool,
            tc.tile_pool(name="vsb", bufs=int(os.environ.get("VSB", 4))) as vsb_pool,
            tc.tile_pool(name="yps", bufs=int(os.environ.get("YPB", 1)), space="PSUM") as yps_pool,
            tc.tile_pool(name="ysb", bufs=2) as ysb_pool,
        ):
            wup_sb = cpool.tile([64, UP_LEN], bf16)
            nc.sync.dma_start(out=wup_sb[:], in_=wup[:])
            wdn_sb = cpool.tile([UP_LEN, W], bf16)
            nc.sync.dma_start(out=wdn_sb[:], in_=wdn[:])
            alpha_sb = cpool.tile([128, 1], f32)
            nc.vector.memset(alpha_sb[:], NEG_SLOPE)

            assert n_groups % DMA_BATCH == 0 or n_groups < DMA_BATCH
            dma_b = min(DMA_BATCH, n_groups)
            xt4 = None
            ysb4 = None

            import contextlib
            rep_ctx = (tc.For_i(0, repeat, 1) if repeat > 1
                       else contextlib.nullcontext())
            with rep_ctx:
                _group_loop(nc, tc, mybir, n_groups, dma_b, locals())

    nc.compile()
    return nc


def _group_loop(nc, tc, mybir, n_groups, dma_b, env):
    f32 = mybir.dt.float32
    bf16 = mybir.dt.bfloat16
    x, out = env["x"], env["out"]
    wup_sb, wdn_sb, alpha_sb = env["wup_sb"], env["wdn_sb"], env["alpha_sb"]
    xpool, t1ps_pool, t1sb_pool = env["xpool"], env["t1ps_pool"], env["t1sb_pool"]
    ups_pool, ssb_pool, vps_pool = env["ups_pool"], env["ssb_pool"], env["vps_pool"]
    vsb_pool, yps_pool, ysb_pool = env["vsb_pool"], env["yps_pool"], env["ysb_pool"]
    xt4 = None
    ysb4 = None
    if True:
            for g in range(n_groups):
                c0 = g * GROUP
                gb = g % dma_b
                # ---- load x for dma_b groups at once (bf16, host layout) ----
                if gb == 0:
                    xt4 = xpool.tile([64, dma_b * GROUP * W], bf16)
                    nc.sync.dma_start(
                        out=xt4[:],
                        in_=x[:, c0 * W:(c0 + dma_b * GROUP) * W])
                    ysb4 = ysb_pool.tile([64, dma_b * GROUP * W], f32)
                xt = xt4[:, gb * GROUP * W:(gb + 1) * GROUP * W]

                # ---- M1: 8 single-plane matmuls -> t1 [64w, 128h'] each ----
                # (matmul operands must live at partitions 0-63 or span all
                #  128 — base-partition-64 operands hang the PE array)
                t1ps = t1ps_pool.tile([64, 1024], f32)
                for p in range(GROUP):
                    nc.tensor.matmul(
                        t1ps[:, p * 128:(p + 1) * 128],
                        lhsT=xt[:, p * 64:(p + 1) * 64],
                        rhs=wup_sb[:],
                        start=True, stop=True,
                    )
                t1sb = t1sb_pool.tile([64, 1024], bf16)
                nc.vector.tensor_copy(out=t1sb[:], in_=t1ps[:])

                # ---- M2: 2 bulk matmuls -> u^T [128w', 128h'] x 8 planes ----
                ups = ups_pool.tile([128, 1024], f32)
                for half in range(2):
                    nc.tensor.matmul(
                        ups[:, half * 512:(half + 1) * 512],
                        lhsT=wup_sb[:],
                        rhs=t1sb[:, half * 512:(half + 1) * 512],
                        start=True, stop=True,
                    )

                # ---- L: lrelu evac PSUM->SBUF bf16 (one wide op) ----
                ssb = ssb_pool.tile([128, 1024], bf16)
                nc.scalar.activation(
                    out=ssb[:],
                    in_=ups[:],
                    func=mybir.ActivationFunctionType.Prelu,
                    scale=SQRT2,
                    alpha=alpha_sb[:],
                )

                # ---- M3: 8 matmuls -> v per plane [128h', 64w''] ----
                vps = vps_pool.tile([128, 512], f32)
                for p in range(GROUP):
                    nc.tensor.matmul(
                        vps[:, p * 64:(p + 1) * 64],
                        lhsT=ssb[:, p * 128:(p + 1) * 128],
                        rhs=wdn_sb[:],
                        start=True, stop=True,
                    )
                vsb = vsb_pool.tile([128, 512], bf16)
                nc.vector.tensor_copy(out=vsb[:], in_=vps[:])

                # ---- M4: one matmul -> y [64h'', 512] ----
                yps = yps_pool.tile([64, 512], f32)
                nc.tensor.matmul(
                    yps[:], lhsT=wdn_sb[:], rhs=vsb[:], start=True, stop=True,
                )
                nc.scalar.activation(
                    out=ysb4[:, gb * GROUP * W:(gb + 1) * GROUP * W],
                    in_=yps[:],
                    func=mybir.ActivationFunctionType.Copy)

                # ---- store dma_b groups at once ----
                if gb == dma_b - 1:
                    nc.sync.dma_start(
                        out=out[:, (c0 - (dma_b - 1) * GROUP) * W:
                                (c0 + GROUP) * W],
                        in_=ysb4[:],
                    )


def kernel(input, bias, up_filter, down_filter):
    global _LAST_RESULT, _CACHED
    from concourse.bass_utils import run_bass_kernel_spmd

    input = np.asarray(input, dtype=np.float32)
    bias = np.asarray(bias, dtype=np.float32)
    if np.any(bias):
        input = input + bias.reshape(1, C, 1, 1)
    # [B, C, H, W] -> per-core [H, C*W] bf16
    x_t = np.ascontiguousarray(
        input.astype(_BF16).transpose(0, 2, 1, 3).reshape(B, H, C * W))

    if _CACHED is None:
        _CACHED = _build_bass()
    nc = _CACHED

    wup_m, wdn_m = _build_consts(up_filter, down_filter)

    in_maps = []
    for i in range(N_CORES):
        in_maps.append({
            "x": x_t[i],
            "wup": wup_m,
            "wdn": wdn_m,
        })

    res = run_bass_kernel_spmd(nc, in_maps, core_ids=list(range(N_CORES)))
    _LAST_RESULT = res
    # per-core [H, C*W] f32 -> [B, C, H, W]
    y = np.stack([r["out"] for r in res.results], axis=0)
    return np.ascontiguousarray(
        y.reshape(B, H, C, W).transpose(0, 2, 1, 3))


# revision 51
# speedup vs baseline: 1.8895x; 1.8895x over previous
"""Trainium2 Bass kernel for nn_AliasFreeActivation (alias-free GAN activation).

Pipeline per (n, c) plane X [64, 64]:
    y = Wdn.T @ ( sqrt(2) * lrelu_0.2( Wup.T @ (X + b) @ Wup ) ) @ Wdn
where Wup [64, 128] / Wdn [128, 64] are the upfirdn band matrices for the
separable 12-tap filter (up=2 / down=2), built on host.

Device mapping (fused matmul chain — zero transposes):
    M1  t1 = x_aug.T @ Wup_aug    [2 planes: 128(2w), 128h']   (data as lhsT)
    M2  u^T = Wup.T @ t1_p        [128w', 128h']               (filter stationary)
    L   s = lrelu(sqrt2 * u)      ACT Lrelu alpha=0.2, PSUM->SBUF bf16
    M3  v = s.T @ Wdn             [128h', 64w'']               (data as lhsT)
    M4  y = Wdn.T @ v_all         [64h'', 512]                 (one matmul / group)

Sharding: pure data parallel over batch: core i gets input[i] -> [512, 64, 64].
Each core processes 64 groups of 8 channel-planes.
"""

import os
import sys

for _p in ("/opt/trn_rl_repo", "/opt/pypackages"):
    if _p not in sys.path:
        sys.path.append(_p)

import numpy as np
import ml_dtypes

N_CORES = 8
B, C, H, W = 8, 512, 64, 64
GROUP = 8                 # channel planes per group
N_GROUPS = C // GROUP     # 64
DMA_BATCH = 4             # groups per DMA transfer
UP_LEN = 128
NEG_SLOPE = 0.2
SQRT2 = float(2.0 ** 0.5)

# 12-tap hann-windowed-sinc lowpass, as in the reference module
_FILT = np.array([0.0, 0.00398, -0.01884, -0.05155, 0.12443, 0.44197,
                  0.44197, 0.12443, -0.05155, -0.01884, 0.00398, 0.0],
                 dtype=np.float64)
_FILT = _FILT / _FILT.sum()

_BF16 = ml_dtypes.bfloat16

_LAST_RESULT = None   # BassKernelResults of the most recent run (for test.py)
_CACHED = None        # (nc, meta) cache so repeat kernel() calls skip rebuild


def _upfirdn_matrix(k, L, up, down, pad0, pad1):
    """Band matrix Wf such that y = x @ Wf applies upfirdn along an axis."""
    K = len(k)
    Ld = (L - 1) * up + 1
    n_out = (Ld + pad0 + (pad1 + up - 1) - K) // down + 1
    Wf = np.zeros((L, n_out), dtype=np.float64)
    for j in range(n_out):
        for t in range(K):
            m = j * down + t - pad0
            if 0 <= m < Ld and m % up == 0:
                Wf[m // up, j] += k[K - 1 - t]
    return Wf


def _build_consts(up_filter, down_filter):
    k_up = np.asarray(up_filter, dtype=np.float64) * 2.0   # prescaled by UP
    k_dn = np.asarray(down_filter, dtype=np.float64)
    Wup = _upfirdn_matrix(k_up, 64, 2, 1, 6, 5)            # [64, 128]
    Wdn = _upfirdn_matrix(k_dn, 128, 1, 2, 5, 5)           # [128, 64]
    # zero-padded K=128 variants: contract only the low/high 64 partitions
    # of a full-128 rhs (dodges the base-partition-64 operand restriction)
    wupz = np.zeros((128, 2 * UP_LEN), dtype=np.float64)
    wupz[0:64, 0:UP_LEN] = Wup        # "even" half: contracts partitions 0-63
    wupz[64:128, UP_LEN:] = Wup       # "odd" half: contracts partitions 64-127
    return Wup.astype(_BF16), Wdn.astype(_BF16), wupz.astype(_BF16)


def _build_bass(n_groups=N_GROUPS, repeat=1):
    import concourse.bacc as bacc
    import concourse.mybir as mybir
    from concourse.tile import TileContext

    f32 = mybir.dt.float32
    bf16 = mybir.dt.bfloat16

    nc = bacc.Bacc("TRN2", target_bir_lowering=False)

    # x / out live in DRAM as [H, C*W] (host pre/post-transposes) so every
    # DMA is a plain 2-D slice with 512*dsize contiguous bytes per partition.
    x = nc.dram_tensor("x", [H, C * W], bf16, kind="ExternalInput")
    wup = nc.dram_tensor("wup", [64, UP_LEN], bf16, kind="ExternalInput")
    wupz = nc.dram_tensor("wupz", [128, 2 * UP_LEN], bf16,
                          kind="ExternalInput")
    wdn = nc.dram_tensor("wdn", [UP_LEN, W], bf16, kind="ExternalInput")
    out = nc.dram_tensor("out", [H, C * W], f32, kind="ExternalOutput")

    with TileContext(nc) as tc:
        with (
            tc.tile_pool(name="consts", bufs=1) as cpool,
            tc.tile_pool(name="xt", bufs=int(os.environ.get("XB", 3))) as xpool,
            tc.tile_pool(name="t1ps", bufs=int(os.environ.get("T1B", 2)), space="PSUM") as t1ps_pool,
            tc.tile_pool(name="t1sb", bufs=int(os.environ.get("T1SB", 4))) as t1sb_pool,
            tc.tile_pool(name="ups", bufs=int(os.environ.get("UPB", 2)), space="PSUM") as ups_pool,
            tc.tile_pool(name="ssb", bufs=int(os.environ.get("SSB", 4))) as ssb_pool,
            tc.tile_pool(name="vps", bufs=int(os.environ.get("VPB", 1)), space="PSUM") as vps_pool,
            tc.tile_pool(name="vsb", bufs=int(os.environ.get("VSB", 4))) as vsb_pool,
            tc.tile_pool(name="yps", bufs=int(os.environ.get("YPB", 1)), space="PSUM") as yps_pool,
            tc.tile_pool(name="ysb", bufs=2) as ysb_pool,
        ):
            wup_sb = cpool.tile([64, UP_LEN], bf16)
            nc.sync.dma_start(out=wup_sb[:], in_=wup[:])
            wupz_sb = cpool.tile([128, 2 * UP_LEN], bf16)
            nc.sync.dma_start(out=wupz_sb[:], in_=wupz[:])
            wdn_sb = cpool.tile([UP_LEN, W], bf16)
            nc.sync.dma_start(out=wdn_sb[:], in_=wdn[:])
            alpha_sb = cpool.tile([128, 1], f32)
            nc.vector.memset(alpha_sb[:], NEG_SLOPE)

            assert n_groups % DMA_BATCH == 0 or n_groups < DMA_BATCH
            dma_b = min(DMA_BATCH, n_groups)
            xt4 = None
            ysb4 = None

            import contextlib
            rep_ctx = (tc.For_i(0, repeat, 1) if repeat > 1
                       else contextlib.nullcontext())
            with rep_ctx:
                _group_loop(nc, tc, mybir, n_groups, dma_b, locals())

    nc.compile()
    return nc


def _group_loop(nc, tc, mybir, n_groups, dma_b, env):
    f32 = mybir.dt.float32
    bf16 = mybir.dt.bfloat16
    x, out = env["x"], env["out"]
    wup_sb, wdn_sb, alpha_sb = env["wup_sb"], env["wdn_sb"], env["alpha_sb"]
    wupz_sb = env["wupz_sb"]
    xpool, t1ps_pool, t1sb_pool = env["xpool"], env["t1ps_pool"], env["t1sb_pool"]
    ups_pool, ssb_pool, vps_pool = env["ups_pool"], env["ssb_pool"], env["vps_pool"]
    vsb_pool, yps_pool, ysb_pool = env["vsb_pool"], env["yps_pool"], env["ysb_pool"]
    xt4 = None
    ysb4 = None
    if True:
            for g in range(n_groups):
                c0 = g * GROUP
                gb = g % dma_b
                # ---- load x for dma_b groups at once (bf16, host layout) ----
                if gb == 0:
                    xt4 = xpool.tile([64, dma_b * GROUP * W], bf16)
                    nc.sync.dma_start(
                        out=xt4[:],
                        in_=x[:, c0 * W:(c0 + dma_b * GROUP) * W])
                    ysb4 = ysb_pool.tile([64, dma_b * GROUP * W], f32)
                xt = xt4[:, gb * GROUP * W:(gb + 1) * GROUP * W]

                # ---- M1: 4 pair matmuls -> t1 pairs [128(2pl w), 128h'] ----
                # (full-128 output partitions; operands at base 0)
                t1ps = t1ps_pool.tile([128, 512], f32)
                for j in range(4):
                    nc.tensor.matmul(
                        t1ps[:, j * 128:(j + 1) * 128],
                        lhsT=xt[:, j * 128:(j + 1) * 128],
                        rhs=wup_sb[:],
                        start=True, stop=True,
                    )
                t1sb = t1sb_pool.tile([128, 512], bf16)
                nc.vector.tensor_copy(out=t1sb[:], in_=t1ps[:])

                # ---- M2: 2 bulk K=128 matmuls with zero-padded weights ----
                # even half contracts partitions 0-63 (even planes of each
                # pair), odd half contracts 64-127 — rhs stays at base 0.
                ups = ups_pool.tile([128, 1024], f32)
                for half in range(2):
                    nc.tensor.matmul(
                        ups[:, half * 512:(half + 1) * 512],
                        lhsT=wupz_sb[:, half * UP_LEN:(half + 1) * UP_LEN],
                        rhs=t1sb[:],
                        start=True, stop=True,
                    )

                # ---- L: lrelu evac PSUM->SBUF bf16 (one wide op) ----
                ssb = ssb_pool.tile([128, 1024], bf16)
                nc.scalar.activation(
                    out=ssb[:],
                    in_=ups[:],
                    func=mybir.ActivationFunctionType.Prelu,
                    scale=SQRT2,
                    alpha=alpha_sb[:],
                )

                # ---- M3: 8 matmuls -> v per plane [128h', 64w''] ----
                # plane p lives at ssb[:, (p%2)*512 + (p//2)*128 : +128]
                vps = vps_pool.tile([128, 512], f32)
                for p in range(GROUP):
                    s_off = (p % 2) * 512 + (p // 2) * 128
                    nc.tensor.matmul(
                        vps[:, p * 64:(p + 1) * 64],
                        lhsT=ssb[:, s_off:s_off + 128],
                        rhs=wdn_sb[:],
                        start=True, stop=True,
                    )
                vsb = vsb_pool.tile([128, 512], bf16)
                nc.vector.tensor_copy(out=vsb[:], in_=vps[:])

                # ---- M4: one matmul -> y [64h'', 512] ----
                yps = yps_pool.tile([64, 512], f32)
                nc.tensor.matmul(
                    yps[:], lhsT=wdn_sb[:], rhs=vsb[:], start=True, stop=True,
                )
                nc.scalar.activation(
                    out=ysb4[:, gb * GROUP * W:(gb + 1) * GROUP * W],
                    in_=yps[:],
                    func=mybir.ActivationFunctionType.Copy)

                # ---- store dma_b groups at once ----
                if gb == dma_b - 1:
                    nc.sync.dma_start(
                        out=out[:, (c0 - (dma_b - 1) * GROUP) * W:
                                (c0 + GROUP) * W],
                        in_=ysb4[:],
                    )


def kernel(input, bias, up_filter, down_filter):
    global _LAST_RESULT, _CACHED
    from concourse.bass_utils import run_bass_kernel_spmd

    input = np.asarray(input, dtype=np.float32)
    bias = np.asarray(bias, dtype=np.float32)
    if np.any(bias):
        input = input + bias.reshape(1, C, 1, 1)
    # [B, C, H, W] -> per-core [H, C*W] bf16
    x_t = np.ascontiguousarray(
        input.astype(_BF16).transpose(0, 2, 1, 3).reshape(B, H, C * W))

    if _CACHED is None:
        _CACHED = _build_bass()
    nc = _CACHED

    wup_m, wdn_m, wupz_m = _build_consts(up_filter, down_filter)

    in_maps = []
    for i in range(N_CORES):
        in_maps.append({
            "x": x_t[i],
            "wup": wup_m,
            "wupz": wupz_m,
            "wdn": wdn_m,
        })

    res = run_bass_kernel_spmd(nc, in_maps, core_ids=list(range(N_CORES)))
    _LAST_RESULT = res
    # per-core [H, C*W] f32 -> [B, C, H, W]
    y = np.stack([r["out"] for r in res.results], axis=0)
    return np.ascontiguousarray(
        y.reshape(B, H, C, W).transpose(0, 2, 1, 3))
